# revision 1
# baseline (speedup 1.0000x reference)
"""Trainium2 Bass kernel for a GNN message-passing layer (GCL).

reference:
    m   = relu(concat(h[row], h[col]) @ edge_w + edge_b)       # [E, H]
    agg = segment_sum(m, row, N)                               # [N, H]
    out = relu(concat(h, agg) @ node_w + node_b)               # [N, H]

Strategy (8 cores, edge-parallel with node-range ownership, no collectives):
  * Precompute AB[n] = [h[n] @ Wtop + b | h[n] @ Wbot]  (bf16, DRAM table).
    Then m_e = relu(AB[row_e].A + AB[col_e].B), a pure gather + add.
  * Sort each core's edges by (col-chunk, dest-window); the dma_gather
    int16-index limit is handled by splitting the col table into 4 chunks.
  * Segment-sum via one-hot matmul: for each 128-edge chunk belonging to one
    128-node window, onehot[e, j] = (row_local[e] == j) built with a DVE
    is_equal against an iota; PE accumulates m.T @ onehot in PSUM.
  * Node MLP with bias folded in via an appended ones-row (K=65 matmul).
Each core owns a contiguous 12544-node range; rows of its edges fall in that
range, so aggregation and the node MLP are fully local.
"""

import math
import numpy as np
import ml_dtypes

import concourse.bass as bass
import concourse.bacc as bacc
import concourse.tile as tile
from concourse import mybir
from concourse.tile import TileContext
from concourse.library_config import mlp as mlp_library

BF16 = mybir.dt.bfloat16
F32 = mybir.dt.float32
I16 = mybir.dt.int16
NP_BF16 = ml_dtypes.bfloat16


class Cfg:
    def __init__(self, n_nodes, n_cores=8, spc=None, table_f32=False):
        self.n_swdge_queues = 4   # parallel SWDGE queues: 8x gather throughput
        self.col_sort = False
        self.N = n_nodes
        self.n_cores = n_cores
        self.NPC = int(math.ceil(n_nodes / n_cores / 128)) * 128
        self.NP = self.NPC * n_cores
        self.W = self.NPC // 128          # windows per core
        self.C = 4                        # col chunks
        assert self.NP % self.C == 0
        self.CHUNK = self.NP // self.C
        assert self.CHUNK <= 32767, "int16 gather index limit"
        if spc is None:
            spc = max(d for d in range(1, 17) if self.W % d == 0 and (self.W // d) % 2 == 0 or d == 1)
        # segments (windows) per gather call; must divide W
        self.SPC = spc
        assert self.W % self.SPC == 0
        self.CALLS_PER_CHUNK = self.W // self.SPC
        # idx loads cover IDX_CALLS gather calls each
        self.IDX_CALLS = self.CALLS_PER_CHUNK // 2 if self.CALLS_PER_CHUNK % 2 == 0 else self.CALLS_PER_CHUNK
        self.table_f32 = table_f32
        self.SEG = None  # set from data

    def stripe(self, total):
        for cand in (8192, 6272, 4096, 3136, 2048, 1792, 1568, 1024, 896, 784, 512, 448, 256, 128):
            if cand <= total and total % cand == 0:
                return cand
        raise AssertionError(total)


def build_kernel(cfg, phases=(0, 1, 2), p1_level=4, p2_level=3):
    """Build the single-core SPMD program. Returns nc.
    p1_level: 1=gathers only, 2=+add/relu, 3=+onehot, 4=full (matmul+flush)."""
    SEG = cfg.SEG
    assert SEG is not None and SEG % 128 == 0
    EP = cfg.C * cfg.W * SEG               # padded edges per core
    NCALL = cfg.SPC * SEG                  # idxs per gather call
    JPC = NCALL // 128                     # 128-chunks per call
    JPS = SEG // 128                       # 128-chunks per segment
    TDT = F32 if cfg.table_f32 else BF16   # gather table dtype
    TESZ = 64 if cfg.table_f32 else 128    # gather elem_size (=256B either way)
    # >64 descriptors/engine in one packet wedges the device; the per-engine
    # descriptor count is NCALL//16 + 1.
    SINGLE_PACKET = (NCALL // 16 + 1) <= 64

    NSWQ = getattr(cfg, "n_swdge_queues", 1)
    nc = bacc.Bacc("TRN2", target_bir_lowering=False, debug=False,
                   num_swdge_queues=NSWQ)

    # ---- DRAM I/O ----
    hTa_d = nc.dram_tensor("hTa", [65, cfg.NP], F32, kind="ExternalInput")
    hTown_d = nc.dram_tensor("hTown", [65, cfg.NPC], F32, kind="ExternalInput")
    waug_d = nc.dram_tensor("waug", [65, 128], F32, kind="ExternalInput")
    nw1_d = nc.dram_tensor("nw1", [64, 64], F32, kind="ExternalInput")
    nw2a_d = nc.dram_tensor("nw2a", [65, 64], F32, kind="ExternalInput")
    iota_d = nc.dram_tensor("iota", [128, 128], BF16, kind="ExternalInput")
    colidx_d = nc.dram_tensor("colidx", [128, EP // 16], I16, kind="ExternalInput")
    rowidx_d = nc.dram_tensor("rowidx", [128, EP // 16], I16, kind="ExternalInput")
    rl_d = nc.dram_tensor("rl", [128, EP // 128], BF16, kind="ExternalInput")
    AB_ds = [nc.dram_tensor(f"AB{c}", [cfg.CHUNK, 128], TDT)
             for c in range(cfg.C)]
    Aown_d = nc.dram_tensor("Aown", [cfg.NPC, 128], TDT)
    out_d = nc.dram_tensor("out", [cfg.NPC, 64], F32, kind="ExternalOutput")

    with TileContext(nc) as tc:
        nc.gpsimd.load_library(mlp_library)

        with tc.tile_pool(name="const", bufs=1) as cpool:
            waug_sb = cpool.tile([65, 128], F32)
            nc.sync.dma_start(out=waug_sb[:], in_=waug_d[:])
            iota_sb = cpool.tile([128, 128], BF16)
            nc.sync.dma_start(out=iota_sb[:], in_=iota_d[:])
            nw1_sb = cpool.tile([64, 64], F32)
            nc.sync.dma_start(out=nw1_sb[:], in_=nw1_d[:])
            nw2a_sb = cpool.tile([65, 64], F32)
            nc.sync.dma_start(out=nw2a_sb[:], in_=nw2a_d[:])

            # aggT arena [65, NPC]: rows 0:64 = aggT, row 64 = ones (bias row)
            arena = cpool.tile([65, cfg.NPC], F32)
            nc.vector.memset(arena[64:65, :], 1.0)

            # ---- Phase 0: build AB table (all NP nodes) and Aown (own nodes) ----
            def ab_pass(src_d, dst, total):
                SN = cfg.stripe(cfg.CHUNK if isinstance(dst, list) else total)
                JT = SN // 128
                with tc.tile_pool(name="p0", bufs=2) as p0, \
                     tc.tile_pool(name="p0ps", bufs=4, space="PSUM") as p0ps:
                    for s in range(total // SN):
                        hstripe = p0.tile([65, SN], F32, tag="hstripe")
                        nc.sync.dma_start(
                            out=hstripe[:], in_=src_d[:, s * SN:(s + 1) * SN])
                        abst = p0.tile([128, JT, TESZ * (2 if cfg.table_f32 else 1)], TDT, tag="abst")
                        for j in range(JT):
                            ps = p0ps.tile([128, 128], F32)
                            nc.tensor.matmul(
                                out=ps[:], lhsT=hstripe[:, j * 128:(j + 1) * 128],
                                rhs=waug_sb[:], start=True, stop=True)
                            nc.vector.tensor_copy(out=abst[:, j, :], in_=ps[:])
                        if isinstance(dst, list):
                            n0 = s * SN
                            dst_d, off = dst[n0 // cfg.CHUNK], n0 % cfg.CHUNK
                        else:
                            dst_d, off = dst, s * SN
                        nc.sync.dma_start(
                            out=dst_d[off:off + SN, :].rearrange(
                                "(j p) f -> p j f", p=128),
                            in_=abst[:])

            if 0 in phases:
                ab_pass(hTown_d, Aown_d, cfg.NPC)
                ab_pass(hTa_d, AB_ds, cfg.NP)

            # ---- Phase 1: gather + edge MLP + one-hot aggregation ----
            if 1 in phases:
              with tc.tile_pool(name="rlp", bufs=1) as rlp:
                rl_sb = rlp.tile([128, EP // 128], BF16)
                nc.sync.dma_start(out=rl_sb[:], in_=rl_d[:])

                with tc.tile_pool(name="idxp", bufs=2) as idxp, \
                     tc.tile_pool(name="gath", bufs=4) as gathp, \
                     tc.tile_pool(name="mp", bufs=3) as mp, \
                     tc.tile_pool(name="ohp", bufs=3) as ohp, \
                     tc.tile_pool(name="p1ps", bufs=4, space="PSUM") as p1ps:
                    IC = cfg.IDX_CALLS
                    ILEN = IC * NCALL // 16      # idx cols per load
                    for c in range(cfg.C):
                        col_tab = AB_ds[c][:, 64:128] if cfg.table_f32 \
                            else AB_ds[c][:]
                        row_tab = Aown_d[:, 0:64] if cfg.table_f32 else Aown_d[:]
                        for g in range(cfg.CALLS_PER_CHUNK // IC):
                            goff = (c * cfg.CALLS_PER_CHUNK + g * IC) * NCALL // 16
                            cidx = idxp.tile([128, ILEN], I16, tag="cidx")
                            nc.sync.dma_start(
                                out=cidx[:], in_=colidx_d[:, goff:goff + ILEN])
                            ridx = idxp.tile([128, ILEN], I16, tag="ridx")
                            nc.sync.dma_start(
                                out=ridx[:], in_=rowidx_d[:, goff:goff + ILEN])
                            for cl in range(IC):
                                call = (c * cfg.CALLS_PER_CHUNK + g * IC + cl)
                                colg = gathp.tile([128, JPC, TESZ], TDT, tag="g")
                                nc.gpsimd.dma_gather(
                                    colg[:], col_tab,
                                    cidx[:, cl * (NCALL // 16):(cl + 1) * (NCALL // 16)],
                                    NCALL, NCALL, TESZ, single_packet=SINGLE_PACKET,
                                    queue_num=(2 * call) % NSWQ,
                                    elem_step=128 if cfg.table_f32 else None)
                                rowg = gathp.tile([128, JPC, TESZ], TDT, tag="g")
                                nc.gpsimd.dma_gather(
                                    rowg[:], row_tab,
                                    ridx[:, cl * (NCALL // 16):(cl + 1) * (NCALL // 16)],
                                    NCALL, NCALL, TESZ, single_packet=SINGLE_PACKET,
                                    queue_num=(2 * call + 1) % NSWQ,
                                    elem_step=128 if cfg.table_f32 else None)
                                for s in range(cfg.SPC if p1_level >= 2 else 0):
                                    w = (call % cfg.CALLS_PER_CHUNK) * cfg.SPC + s
                                    gseg = call * cfg.SPC + s
                                    j0 = s * JPS
                                    a_sl = rowg[:, j0:j0 + JPS, 0:64]
                                    b_sl = colg[:, j0:j0 + JPS, 64:128] if not cfg.table_f32 \
                                        else colg[:, j0:j0 + JPS, 0:64]
                                    madd = mp.tile([128, JPS, 64], TDT, tag="madd")
                                    nc.vector.tensor_tensor(
                                        out=madd[:], in0=a_sl, in1=b_sl,
                                        op=mybir.AluOpType.add)
                                    m2 = mp.tile([128, JPS, 64], TDT, tag="m2")
                                    nc.scalar.activation(
                                        out=m2[:], in_=madd[:],
                                        func=mybir.ActivationFunctionType.Relu)
                                    if p1_level < 3:
                                        continue
                                    oh = ohp.tile([128, JPS, 128], TDT, tag="oh")
                                    nc.vector.tensor_tensor(
                                        out=oh[:],
                                        in0=rl_sb[:, gseg * JPS:gseg * JPS + JPS]
                                            .to_broadcast([128, JPS, 128]),
                                        in1=iota_sb[:].rearrange("p (a b) -> p a b", a=1)
                                            .to_broadcast([128, JPS, 128]),
                                        op=mybir.AluOpType.is_equal)
                                    if p1_level < 4:
                                        continue
                                    ps = p1ps.tile([64, 128], F32)
                                    for j in range(JPS):
                                        nc.tensor.matmul(
                                            out=ps[:], lhsT=m2[:, j, :], rhs=oh[:, j, :],
                                            start=(j == 0), stop=(j == JPS - 1))
                                    dst = arena[0:64, w * 128:(w + 1) * 128]
                                    if c == 0:
                                        nc.vector.tensor_copy(out=dst, in_=ps[:])
                                    else:
                                        nc.vector.tensor_tensor(
                                            out=dst, in0=dst, in1=ps[:],
                                            op=mybir.AluOpType.add)

            # ---- Phase 2: node MLP ----
            if 2 in phases:
              with tc.tile_pool(name="p2", bufs=2) as p2, \
                 tc.tile_pool(name="p2ps", bufs=4, space="PSUM") as p2ps:
                GW = cfg.SPC                  # windows per output group
                for g in range(cfg.W // GW):
                    hT2 = p2.tile([65, GW * 128], F32, tag="hT2")
                    nc.sync.dma_start(
                        out=hT2[:],
                        in_=hTown_d[:, g * GW * 128:(g + 1) * GW * 128])
                    ost = p2.tile([128, GW, 64], F32, tag="ost")
                    for i in range(GW):
                        w = g * GW + i
                        if p2_level < 2:
                            nc.vector.memset(ost[:, i, :], 0.0)
                            continue
                        ps = p2ps.tile([128, 64], F32)
                        nc.tensor.matmul(
                            out=ps[:], lhsT=hT2[0:64, i * 128:(i + 1) * 128],
                            rhs=nw1_sb[:], start=True, stop=False)
                        nc.tensor.matmul(
                            out=ps[:], lhsT=arena[:, w * 128:(w + 1) * 128],
                            rhs=nw2a_sb[:], start=False, stop=True)
                        if p2_level < 3:
                            nc.vector.tensor_copy(out=ost[:, i, :], in_=ps[:])
                        else:
                            nc.scalar.activation(
                                out=ost[:, i, :], in_=ps[:],
                                func=mybir.ActivationFunctionType.Relu)
                    nc.sync.dma_start(
                        out=out_d[g * GW * 128:(g + 1) * GW * 128, :].rearrange(
                            "(j p) f -> p j f", p=128),
                        in_=ost[:])

    nc.compile()
    return nc


# ---------------- host-side data prep ----------------

def _wrap16(a):
    x = np.ascontiguousarray(a.reshape(-1, 16).T)
    return np.tile(x, (8, 1))


def _wrap128(a):
    return np.ascontiguousarray(a.reshape(-1, 128).T)


def prep_inputs(cfg, h, edge_index, edge_w, edge_b, node_w, node_b):
    """Returns (in_maps, SEG). Sets cfg.SEG."""
    N = cfg.N
    row = np.asarray(edge_index[0])
    col = np.asarray(edge_index[1])
    h = np.asarray(h, dtype=np.float32)

    # hT augmented with ones row, padded to NP cols
    hTa = np.zeros((65, cfg.NP), np.float32)
    hTa[:64, :N] = h.T
    hTa[64, :] = 1.0

    waug = np.zeros((65, 128), np.float32)
    waug[:64, 0:64] = edge_w[:64]
    waug[:64, 64:128] = edge_w[64:]
    waug[64, 0:64] = edge_b

    nw1 = np.ascontiguousarray(node_w[:64], dtype=np.float32)
    nw2a = np.concatenate([node_w[64:], node_b[None, :]], axis=0).astype(np.float32)

    iota = np.tile(np.arange(128, dtype=np.float32), (128, 1)).astype(NP_BF16)

    # per-core edge prep; SEG = global max segment length (uniform program)
    per_core = []
    maxc = 1
    for k in range(cfg.n_cores):
        base = k * cfg.NPC
        m = (row >= base) & (row < base + cfg.NPC)
        r = (row[m] - base).astype(np.int64)
        c = col[m].astype(np.int64)
        w = r >> 7
        cc = c // cfg.CHUNK
        seg_id = cc * cfg.W + w
        if getattr(cfg, "col_sort", False):
            # ascending cols inside each segment (HBM locality A/B knob)
            order = np.lexsort((c, seg_id))
        else:
            order = np.argsort(seg_id, kind="stable")
        r, c, seg_id = r[order], c[order], seg_id[order]
        counts = np.bincount(seg_id, minlength=cfg.C * cfg.W)
        if counts.size and r.size:
            maxc = max(maxc, int(counts.max()))
        per_core.append((r, c, seg_id, counts))
    SEG = int(math.ceil(maxc / 128.0)) * 128
    cfg.SEG = SEG
    EP = cfg.C * cfg.W * SEG

    in_maps = []
    for k in range(cfg.n_cores):
        r, c, seg_id, counts = per_core[k]
        starts = np.cumsum(counts) - counts
        intra = np.arange(r.size) - np.repeat(starts, counts)
        slots = seg_id * SEG + intra
        colidx = np.zeros(EP, np.int16)
        rowidx = np.zeros(EP, np.int16)
        rl = np.full(EP, 255.0, NP_BF16)
        colidx[slots] = (c - (c // cfg.CHUNK) * cfg.CHUNK).astype(np.int16)
        rowidx[slots] = r.astype(np.int16)
        rl[slots] = (r & 127).astype(NP_BF16)

        base = k * cfg.NPC
        hTown = np.ascontiguousarray(hTa[:, base:base + cfg.NPC])
        in_maps.append({
            "hTa": hTa,
            "hTown": hTown,
            "waug": waug,
            "nw1": nw1,
            "nw2a": nw2a,
            "iota": iota,
            "colidx": _wrap16(colidx),
            "rowidx": _wrap16(rowidx),
            "rl": _wrap128(rl),
        })
    return in_maps


def unshard_output(cfg, results):
    outs = [np.asarray(res["out"]) for res in results]
    full = np.concatenate(outs, axis=0)
    return np.ascontiguousarray(full[:cfg.N]).astype(np.float32)


# ---------------- entry point ----------------

def kernel(h, edge_index, edge_w, edge_b, node_w, node_b):
    from concourse.bass_utils import run_bass_kernel_spmd
    cfg = Cfg(n_nodes=100000, n_cores=8, spc=2)
    in_maps = prep_inputs(cfg, h, edge_index, edge_w, edge_b, node_w, node_b)
    nc = build_kernel(cfg)
    res = run_bass_kernel_spmd(nc, in_maps, core_ids=list(range(cfg.n_cores)))
    return unshard_output(cfg, res.results)



# revision 2
# speedup vs baseline: 1.4130x; 1.4130x over previous
"""Trainium2 Bass kernel for a GNN message-passing layer (GCL) — v3.

reference:
    m   = relu(concat(h[row], h[col]) @ edge_w + edge_b)       # [E, H]
    agg = segment_sum(m, row, N)                               # [N, H]
    out = relu(concat(h, agg) @ node_w + node_b)               # [N, H]

Single col-side gather (256B/edge); the row side is reconstructed on PE:
per 128-edge chunk, ohT = PE-transpose(onehot(row_local)), then
rv = ohT.T @ A_window and colg is accumulated into the same PSUM via an
identity-stationary matmul; ACT relu drains PSUM->SBUF. Aggregation is
m.T @ onehot accumulated across a window's 4 col-chunk segments in one
PSUM group (w-major segment order), flushed once per window, with the
node MLP for that window folded into the same pipeline. Work is batched
in global 8-chunk tiles; a 3-segment software-pipeline skew keeps PE
from stalling on DVE/ACT results.
"""

import math
import numpy as np
import ml_dtypes

import concourse.bass as bass
import concourse.bacc as bacc
import concourse.tile as tile
from concourse import mybir
from concourse.tile import TileContext
from concourse.library_config import mlp as mlp_library

BF16 = mybir.dt.bfloat16
F32 = mybir.dt.float32
I16 = mybir.dt.int16
NP_BF16 = ml_dtypes.bfloat16


class Cfg:
    def __init__(self, n_nodes, n_cores=8):
        self.n_swdge_queues = 4
        self.col_sort = True
        self.N = n_nodes
        self.n_cores = n_cores
        self.NPC = int(math.ceil(n_nodes / n_cores / 128)) * 128
        self.NP = self.NPC * n_cores
        self.W = self.NPC // 128          # windows per core
        self.C = 4                        # col chunks
        assert self.NP % self.C == 0
        self.CHUNK = self.NP // self.C
        assert self.CHUNK <= 32767, "int16 gather index limit"
        self.SEG = None  # set from data

    def stripe(self, total):
        for cand in (8192, 6272, 4096, 3136, 2048, 1792, 1568, 1024, 896,
                     784, 512, 448, 256, 128):
            if cand <= total and total % cand == 0:
                return cand
        raise AssertionError(total)


def build_kernel(cfg, phases=(0, 1, 2), p1_level=4):
    SEG = cfg.SEG
    assert SEG is not None and SEG % 128 == 0
    S = cfg.C * cfg.W                      # number of segments
    EP = S * SEG                           # padded edges per core
    JPS = SEG // 128                       # 128-chunks per segment
    assert JPS >= 7, "pipeline skew assumes batches trail by <1 segment"
    NCHUNK = S * JPS                       # total chunks
    NB = (NCHUNK + 7) // 8                 # 8-chunk batches
    SINGLE_PACKET = (SEG // 16 + 1) <= 64
    GSEG = 16                              # segments per cidx load
    ILEN = GSEG * SEG // 16

    NSWQ = cfg.n_swdge_queues
    nc = bacc.Bacc("TRN2", target_bir_lowering=False, debug=False,
                   num_swdge_queues=NSWQ)

    # ---- DRAM I/O ----
    hTa_d = nc.dram_tensor("hTa", [65, cfg.NP], BF16, kind="ExternalInput")
    hTown_d = nc.dram_tensor("hTown", [65, cfg.NPC], BF16, kind="ExternalInput")
    waug_d = nc.dram_tensor("waug", [65, 128], BF16, kind="ExternalInput")
    nw1_d = nc.dram_tensor("nw1", [64, 64], BF16, kind="ExternalInput")
    nw2a_d = nc.dram_tensor("nw2a", [65, 64], F32, kind="ExternalInput")
    iota_d = nc.dram_tensor("iota", [128, 128], BF16, kind="ExternalInput")
    ident_d = nc.dram_tensor("ident", [128, 128], BF16, kind="ExternalInput")
    colidx_d = nc.dram_tensor("colidx", [128, EP // 16], I16, kind="ExternalInput")
    rl_d = nc.dram_tensor("rl", [128, EP // 128], BF16, kind="ExternalInput")
    AB_ds = [nc.dram_tensor(f"AB{c}", [cfg.CHUNK, 128], BF16)
             for c in range(cfg.C)]
    out_d = nc.dram_tensor("out", [cfg.NPC, 64], F32, kind="ExternalOutput")

    with TileContext(nc) as tc:
        nc.gpsimd.load_library(mlp_library)

        with tc.tile_pool(name="const", bufs=1) as cpool:
            waug_sb = cpool.tile([65, 128], BF16)
            nc.sync.dma_start(out=waug_sb[:], in_=waug_d[:])
            iota_sb = cpool.tile([128, 128], BF16)
            nc.sync.dma_start(out=iota_sb[:], in_=iota_d[:])
            ident_sb = cpool.tile([128, 128], BF16)
            nc.sync.dma_start(out=ident_sb[:], in_=ident_d[:])
            nw1_sb = cpool.tile([64, 64], BF16)
            nc.sync.dma_start(out=nw1_sb[:], in_=nw1_d[:])
            nw2a_sb = cpool.tile([65, 64], F32)
            nc.sync.dma_start(out=nw2a_sb[:], in_=nw2a_d[:])

            hTown_sb = cpool.tile([65, cfg.NPC], BF16)
            nc.sync.dma_start(out=hTown_sb[:], in_=hTown_d[:])

            aown_sb = cpool.tile([128, cfg.W, 64], BF16)
            arena = cpool.tile([65, cfg.NPC], F32)
            nc.vector.memset(arena[64:65, :], 1.0)

            # ---- Phase 0a: Aown into SBUF ----
            if 0 in phases:
                with tc.tile_pool(name="p0aps", bufs=4, space="PSUM") as p0aps:
                    for wb in range(cfg.W // 2):
                        ps = p0aps.tile([128, 2, 128], F32)
                        for i in range(2):
                            w = wb * 2 + i
                            nc.tensor.matmul(
                                out=ps[:, i, :],
                                lhsT=hTown_sb[:, w * 128:(w + 1) * 128],
                                rhs=waug_sb[:], start=True, stop=True)
                        if wb % 2 == 0:
                            nc.vector.tensor_copy(
                                out=aown_sb[:, wb * 2:wb * 2 + 2, :],
                                in_=ps[:, :, 0:64])
                        else:
                            nc.scalar.activation(
                                out=aown_sb[:, wb * 2:wb * 2 + 2, :],
                                in_=ps[:, :, 0:64],
                                func=mybir.ActivationFunctionType.Copy)

                # ---- Phase 0b: AB table (all NP nodes) to DRAM ----
                SN = cfg.stripe(cfg.CHUNK)
                JT = SN // 128
                with tc.tile_pool(name="p0", bufs=2) as p0, \
                     tc.tile_pool(name="p0ps", bufs=4, space="PSUM") as p0ps:
                    for st in range(cfg.NP // SN):
                        hstripe = p0.tile([65, SN], BF16, tag="hstripe")
                        nc.sync.dma_start(
                            out=hstripe[:], in_=hTa_d[:, st * SN:(st + 1) * SN])
                        abst = p0.tile([128, JT, 128], BF16, tag="abst")
                        for jb in range((JT + 3) // 4):
                            n = min(4, JT - jb * 4)
                            ps = p0ps.tile([128, 4, 128], F32)
                            for i in range(n):
                                j = jb * 4 + i
                                nc.tensor.matmul(
                                    out=ps[:, i, :],
                                    lhsT=hstripe[:, j * 128:(j + 1) * 128],
                                    rhs=waug_sb[:], start=True, stop=True)
                            if jb % 2 == 0:
                                nc.vector.tensor_copy(
                                    out=abst[:, jb * 4:jb * 4 + n, :],
                                    in_=ps[:, 0:n, :])
                            else:
                                nc.scalar.activation(
                                    out=abst[:, jb * 4:jb * 4 + n, :],
                                    in_=ps[:, 0:n, :],
                                    func=mybir.ActivationFunctionType.Copy)
                        n0 = st * SN
                        dst_d, off = AB_ds[n0 // cfg.CHUNK], n0 % cfg.CHUNK
                        # hTa columns are host-permuted so node off+p*JT+j is
                        # abst[p, j]; per-partition runs are contiguous
                        nc.sync.dma_start(
                            out=dst_d[off:off + SN, :].rearrange(
                                "(p j) f -> p j f", p=128),
                            in_=abst[:])

            # ---- Phase 1 + fused phase 2: skewed pipeline over segments ----
            if 1 in phases:
              with tc.tile_pool(name="rlp", bufs=1) as rlp:
                rl_sb = rlp.tile([128, EP // 128], BF16)
                nc.sync.dma_start(out=rl_sb[:], in_=rl_d[:])

                with tc.tile_pool(name="idxp", bufs=2) as idxp, \
                     tc.tile_pool(name="gath", bufs=4) as gathp, \
                     tc.tile_pool(name="ohp", bufs=6) as ohp, \
                     tc.tile_pool(name="ohtp", bufs=4) as ohtp, \
                     tc.tile_pool(name="mp", bufs=4) as mp, \
                     tc.tile_pool(name="p2o", bufs=2) as p2o, \
                     tc.tile_pool(name="tps", bufs=2, space="PSUM") as tps, \
                     tc.tile_pool(name="rvps", bufs=2, space="PSUM") as rvps, \
                     tc.tile_pool(name="aggps", bufs=2, space="PSUM") as aggps, \
                     tc.tile_pool(name="p2ps", bufs=1, space="PSUM") as p2ps:

                    cidx_tiles = {}
                    oh_tiles = {}          # seg -> tile
                    colg_tiles = {}        # seg -> tile
                    tb_tiles = {}          # batch -> psum tile
                    oht_tiles = {}         # batch -> sbuf tile
                    rv_tiles = {}          # batch -> psum tile
                    m2_tiles = {}          # batch -> sbuf tile
                    agg_tiles = {}         # window -> psum tile
                    n_T = [0]              # chunks transposed so far
                    n_rv = [0]             # chunks rv+madd'ed so far
                    n_relu = [0]           # batches relu'ed so far
                    ost_tiles = {}

                    def load_cidx(g):
                        if g * GSEG >= S:
                            return
                        n = min(GSEG, S - g * GSEG) * SEG // 16
                        t = idxp.tile([128, ILEN], I16, tag="cidx")
                        nc.sync.dma_start(
                            out=t[:, 0:n], in_=colidx_d[:, g * ILEN:g * ILEN + n])
                        cidx_tiles[g] = t

                    def gather(s):
                        g, r = divmod(s, GSEG)
                        t = gathp.tile([128, JPS, 128], BF16, tag="g")
                        nc.gpsimd.dma_gather(
                            t[:], AB_ds[s % cfg.C][:],
                            cidx_tiles[g][:, r * (SEG // 16):(r + 1) * (SEG // 16)],
                            SEG, SEG, 128, single_packet=SINGLE_PACKET,
                            queue_num=s % NSWQ)
                        colg_tiles[s] = t

                    def build_oh(s):
                        t = ohp.tile([128, JPS, 128], BF16, tag="oh")
                        nc.vector.tensor_tensor(
                            out=t[:],
                            in0=rl_sb[:, s * JPS:(s + 1) * JPS]
                                .to_broadcast([128, JPS, 128]),
                            in1=iota_sb[:].rearrange("p (a b) -> p a b", a=1)
                                .to_broadcast([128, JPS, 128]),
                            op=mybir.AluOpType.is_equal)
                        oh_tiles[s] = t

                    def transpose_upto(klim):
                        # transpose chunks [n_T, klim) into batch psum tiles;
                        # drain every completed batch (alternate DVE/ACT)
                        while n_T[0] < klim:
                            k = n_T[0]
                            b = k // 8
                            if k % 8 == 0:
                                tb_tiles[b] = tps.tile([128, 8, 128], BF16,
                                                       tag="tb", name="tb")
                            s, j = divmod(k, JPS)
                            nc.tensor.transpose(
                                out=tb_tiles[b][:, k % 8, :],
                                in_=oh_tiles[s][:, j, :],
                                identity=ident_sb[:])
                            n_T[0] += 1
                            if n_T[0] % 8 == 0 or n_T[0] == NCHUNK:
                                nb_ = 8 if n_T[0] % 8 == 0 else n_T[0] % 8
                                dst = ohtp.tile([128, 8, 128], BF16, tag="ohT")
                                if b % 2 == 0:
                                    nc.vector.tensor_copy(
                                        out=dst[:, 0:nb_, :],
                                        in_=tb_tiles[b][:, 0:nb_, :])
                                else:
                                    nc.scalar.activation(
                                        out=dst[:, 0:nb_, :],
                                        in_=tb_tiles[b][:, 0:nb_, :],
                                        func=mybir.ActivationFunctionType.Copy)
                                oht_tiles[b] = dst
                                tb_tiles.pop(b)

                    def rv_madd_seg(s):
                        # rv + colg accumulate for all chunks of segment s;
                        # relu every batch that becomes fully filled
                        w = s // cfg.C
                        for j in range(JPS):
                            k = s * JPS + j
                            b = k // 8
                            if k % 8 == 0:
                                rv_tiles[b] = rvps.tile([128, 8, 64], F32,
                                                        tag="rv", name="rv")
                            nc.tensor.matmul(
                                out=rv_tiles[b][:, k % 8, :],
                                lhsT=oht_tiles[b][:, k % 8, :],
                                rhs=aown_sb[:, w, :], start=True, stop=False)
                            nc.tensor.matmul(
                                out=rv_tiles[b][:, k % 8, :],
                                lhsT=ident_sb[:],
                                rhs=colg_tiles[s][:, j, 64:128],
                                start=False, stop=True)
                            n_rv[0] += 1
                        colg_tiles.pop(s)
                        while (n_relu[0] + 1) * 8 <= n_rv[0] or \
                              n_rv[0] == NCHUNK and n_relu[0] < NB:
                            b = n_relu[0]
                            nb_ = min(8, NCHUNK - b * 8)
                            dst = mp.tile([128, 8, 64], BF16, tag="m2")
                            nc.scalar.activation(
                                out=dst[:, 0:nb_, :],
                                in_=rv_tiles[b][:, 0:nb_, :],
                                func=mybir.ActivationFunctionType.Relu)
                            m2_tiles[b] = dst
                            rv_tiles.pop(b)
                            n_relu[0] += 1

                    def agg_seg(s):
                        w, c = divmod(s, cfg.C)
                        if c == 0:
                            agg_tiles[w] = aggps.tile([64, 128], F32, tag="agg", name="agg")
                        for j in range(JPS):
                            k = s * JPS + j
                            b = k // 8
                            nc.tensor.matmul(
                                out=agg_tiles[w][:],
                                lhsT=m2_tiles[b][:, k % 8, :],
                                rhs=oh_tiles[s][:, j, :],
                                start=(c == 0 and j == 0),
                                stop=(c == cfg.C - 1 and j == JPS - 1))
                        oh_tiles.pop(s)
                        if (s * JPS + JPS) % 8 != 0:
                            pass  # m2 batches span segments; popped lazily
                        for b in [bb for bb in m2_tiles
                                  if (bb + 1) * 8 <= s * JPS + JPS]:
                            m2_tiles.pop(b)
                        if c == cfg.C - 1:
                            flush_window(w)

                    def flush_window(w):
                        nc.vector.tensor_copy(
                            out=arena[0:64, w * 128:(w + 1) * 128],
                            in_=agg_tiles[w][:])
                        agg_tiles.pop(w)
                        if 2 in phases:
                            node_mlp(w)

                    def node_mlp(w):
                        if w % 2 == 0:
                            ost_tiles[w // 2] = p2o.tile([128, 2, 64], F32,
                                                         tag="ost", name="ost")
                        ps = p2ps.tile([128, 64], F32)
                        nc.tensor.matmul(
                            out=ps[:], lhsT=hTown_sb[0:64, w * 128:(w + 1) * 128],
                            rhs=nw1_sb[:], start=True, stop=False)
                        nc.tensor.matmul(
                            out=ps[:], lhsT=arena[:, w * 128:(w + 1) * 128],
                            rhs=nw2a_sb[:], start=False, stop=True)
                        nc.scalar.activation(
                            out=ost_tiles[w // 2][:, w % 2, :], in_=ps[:],
                            func=mybir.ActivationFunctionType.Relu)
                        if w % 2 == 1:
                            # row p*2+i of the block holds node (w-1+i)*128+p;
                            # unshard_output inverts this on host
                            nc.sync.dma_start(
                                out=out_d[(w - 1) * 128:(w + 1) * 128, :]
                                    .rearrange("(p i) f -> p i f", p=128),
                                in_=ost_tiles[w // 2][:])
                            ost_tiles.pop(w // 2)

                    # ---- the skewed main loop ----
                    load_cidx(0)
                    load_cidx(1)
                    for s in range(S + 3):
                        if s < S and s % GSEG == 0 and s > 0:
                            load_cidx(s // GSEG + 1)
                        if s < S:
                            gather(s)
                        if s < S:
                            build_oh(s)
                        if p1_level >= 2:
                            if s >= 1:
                                transpose_upto(min((s - 1 + 1) * JPS, NCHUNK))
                            if s >= 2 and s - 2 < S:
                                rv_madd_seg(s - 2)
                            if p1_level >= 4 and s >= 3 and s - 3 < S:
                                agg_seg(s - 3)

            elif 2 in phases:
                # standalone phase 2 (for ablation)
                with tc.tile_pool(name="p2", bufs=2) as p2, \
                     tc.tile_pool(name="p2ps", bufs=4, space="PSUM") as p2ps:
                    for g in range(cfg.W // 2):
                        ost = p2.tile([128, 2, 64], F32, tag="ost")
                        for i in range(2):
                            w = g * 2 + i
                            ps = p2ps.tile([128, 64], F32)
                            nc.tensor.matmul(
                                out=ps[:],
                                lhsT=hTown_sb[0:64, w * 128:(w + 1) * 128],
                                rhs=nw1_sb[:], start=True, stop=False)
                            nc.tensor.matmul(
                                out=ps[:], lhsT=arena[:, w * 128:(w + 1) * 128],
                                rhs=nw2a_sb[:], start=False, stop=True)
                            nc.scalar.activation(
                                out=ost[:, i, :], in_=ps[:],
                                func=mybir.ActivationFunctionType.Relu)
                        nc.sync.dma_start(
                            out=out_d[g * 256:(g + 1) * 256, :].rearrange(
                                "(p i) f -> p i f", p=128),
                            in_=ost[:])

    nc.compile()
    return nc


# ---------------- host-side data prep ----------------

def _wrap16(a):
    x = np.ascontiguousarray(a.reshape(-1, 16).T)
    return np.tile(x, (8, 1))


def _wrap128(a):
    return np.ascontiguousarray(a.reshape(-1, 128).T)


def prep_inputs(cfg, h, edge_index, edge_w, edge_b, node_w, node_b):
    N = cfg.N
    row = np.asarray(edge_index[0])
    col = np.asarray(edge_index[1])
    h = np.asarray(h, dtype=np.float32)

    hTa = np.zeros((65, cfg.NP), NP_BF16)
    hTa[:64, :N] = h.T.astype(NP_BF16)
    hTa[64, :] = 1.0
    # permute columns so p0b's natural (p, j) write order lands nodes
    # contiguously per partition: stripe col j*128+p <- node st*SN + p*JT + j
    SN = cfg.stripe(cfg.CHUNK)
    JT = SN // 128
    hTa_p = hTa.reshape(65, cfg.NP // SN, 128, JT).swapaxes(2, 3) \
        .reshape(65, cfg.NP)

    waug = np.zeros((65, 128), NP_BF16)
    waug[:64, 0:64] = edge_w[:64].astype(NP_BF16)
    waug[:64, 64:128] = edge_w[64:].astype(NP_BF16)
    waug[64, 0:64] = edge_b.astype(NP_BF16)

    nw1 = np.ascontiguousarray(node_w[:64]).astype(NP_BF16)
    nw2a = np.concatenate([node_w[64:], node_b[None, :]],
                          axis=0).astype(np.float32)

    iota = np.tile(np.arange(128, dtype=np.float32), (128, 1)).astype(NP_BF16)
    ident = np.eye(128, dtype=np.float32).astype(NP_BF16)

    # per-core edge prep; w-major segment order: seg_id = w*C + cc
    per_core = []
    maxc = 1
    for k in range(cfg.n_cores):
        base = k * cfg.NPC
        m = (row >= base) & (row < base + cfg.NPC)
        r = (row[m] - base).astype(np.int64)
        c = col[m].astype(np.int64)
        w = r >> 7
        cc = c // cfg.CHUNK
        seg_id = w * cfg.C + cc
        if cfg.col_sort:
            order = np.lexsort((c, seg_id))
        else:
            order = np.argsort(seg_id, kind="stable")
        r, c, seg_id = r[order], c[order], seg_id[order]
        counts = np.bincount(seg_id, minlength=cfg.C * cfg.W)
        if counts.size and r.size:
            maxc = max(maxc, int(counts.max()))
        per_core.append((r, c, seg_id, counts))
    SEG = int(math.ceil(maxc / 128.0)) * 128
    cfg.SEG = SEG
    EP = cfg.C * cfg.W * SEG

    in_maps = []
    for k in range(cfg.n_cores):
        r, c, seg_id, counts = per_core[k]
        starts = np.cumsum(counts) - counts
        intra = np.arange(r.size) - np.repeat(starts, counts)
        slots = seg_id * SEG + intra
        colidx = np.zeros(EP, np.int16)
        rl = np.full(EP, 255.0, NP_BF16)
        colidx[slots] = (c - (c // cfg.CHUNK) * cfg.CHUNK).astype(np.int16)
        rl[slots] = (r & 127).astype(NP_BF16)

        base = k * cfg.NPC
        hTown = np.ascontiguousarray(hTa[:, base:base + cfg.NPC])
        in_maps.append({
            "hTa": hTa_p,
            "hTown": hTown,
            "waug": waug,
            "nw1": nw1,
            "nw2a": nw2a,
            "iota": iota,
            "ident": ident,
            "colidx": _wrap16(colidx),
            "rl": _wrap128(rl),
        })
    return in_maps


def unshard_output(cfg, results):
    outs = []
    for res in results:
        o = np.asarray(res["out"])                      # [NPC, 64] permuted
        o = o.reshape(-1, 128, 2, 64).swapaxes(1, 2).reshape(-1, 64)
        outs.append(o)
    full = np.concatenate(outs, axis=0)
    return np.ascontiguousarray(full[:cfg.N]).astype(np.float32)


# ---------------- entry point ----------------

def kernel(h, edge_index, edge_w, edge_b, node_w, node_b):
    from concourse.bass_utils import run_bass_kernel_spmd
    cfg = Cfg(n_nodes=100000, n_cores=8)
    in_maps = prep_inputs(cfg, h, edge_index, edge_w, edge_b, node_w, node_b)
    nc = build_kernel(cfg)
    res = run_bass_kernel_spmd(nc, in_maps, core_ids=list(range(cfg.n_cores)))
    return unshard_output(cfg, res.results)


# revision 3
# speedup vs baseline: 2.8435x; 2.0124x over previous
"""Trainium2 Bass kernel for a GNN message-passing layer (GCL) — v3.

reference:
    m   = relu(concat(h[row], h[col]) @ edge_w + edge_b)       # [E, H]
    agg = segment_sum(m, row, N)                               # [N, H]
    out = relu(concat(h, agg) @ node_w + node_b)               # [N, H]

Single col-side gather (256B/edge); the row side is reconstructed on PE:
per 128-edge chunk, ohT = PE-transpose(onehot(row_local)), then
rv = ohT.T @ A_window and colg is accumulated into the same PSUM via an
identity-stationary matmul; ACT relu drains PSUM->SBUF. Aggregation is
m.T @ onehot accumulated across a window's 4 col-chunk segments in one
PSUM group (w-major segment order), flushed once per window, with the
node MLP for that window folded into the same pipeline. Work is batched
in global 8-chunk tiles; a 3-segment software-pipeline skew keeps PE
from stalling on DVE/ACT results.
"""

import math
import numpy as np
import ml_dtypes

import concourse.bass as bass
import concourse.bacc as bacc
import concourse.tile as tile
from concourse import mybir
from concourse.tile import TileContext
from concourse.library_config import mlp as mlp_library

BF16 = mybir.dt.bfloat16
F32 = mybir.dt.float32
I16 = mybir.dt.int16
NP_BF16 = ml_dtypes.bfloat16


class Cfg:
    def __init__(self, n_nodes, n_cores=8):
        self.n_swdge_queues = 4
        self.col_sort = True
        self.N = n_nodes
        self.n_cores = n_cores
        self.NPC = int(math.ceil(n_nodes / n_cores / 128)) * 128
        self.NP = self.NPC * n_cores
        self.W = self.NPC // 128          # windows per core
        self.C = 4                        # col chunks
        assert self.NP % self.C == 0
        self.CHUNK = self.NP // self.C
        assert self.CHUNK <= 32767, "int16 gather index limit"
        self.SEG = None  # set from data

    def stripe(self, total):
        for cand in (8192, 6272, 4096, 3136, 2048, 1792, 1568, 1024, 896,
                     784, 512, 448, 256, 128):
            if cand <= total and total % cand == 0:
                return cand
        raise AssertionError(total)


def build_kernel(cfg, phases=(0, 1, 2), p1_level=4):
    SEG = cfg.SEG
    assert SEG is not None and SEG % 128 == 0
    S = cfg.C * cfg.W                      # number of segments
    EP = S * SEG                           # padded edges per core
    JPS = SEG // 128                       # 128-chunks per segment
    assert JPS >= 7, "pipeline skew assumes batches trail by <1 segment"
    NCHUNK = S * JPS                       # total chunks
    NB = (NCHUNK + 7) // 8                 # 8-chunk batches
    SINGLE_PACKET = (SEG // 16 + 1) <= 64
    GSEG = 32                              # segments per cidx load
    ILEN = GSEG * SEG // 16

    NSWQ = cfg.n_swdge_queues
    nc = bacc.Bacc("TRN2", target_bir_lowering=False, debug=False,
                   num_swdge_queues=NSWQ)

    # ---- DRAM I/O ----
    hTa_d = nc.dram_tensor("hTa", [65, cfg.NP], BF16, kind="ExternalInput")
    hTown_d = nc.dram_tensor("hTown", [65, cfg.NPC], BF16, kind="ExternalInput")
    waug_d = nc.dram_tensor("waug", [65, 128], BF16, kind="ExternalInput")
    nw1_d = nc.dram_tensor("nw1", [64, 64], BF16, kind="ExternalInput")
    nw2a_d = nc.dram_tensor("nw2a", [65, 64], F32, kind="ExternalInput")
    iota_d = nc.dram_tensor("iota", [128, 128], BF16, kind="ExternalInput")
    ident_d = nc.dram_tensor("ident", [128, 128], BF16, kind="ExternalInput")
    colidx_d = nc.dram_tensor("colidx", [128, EP // 16], I16, kind="ExternalInput")
    rl_d = nc.dram_tensor("rl", [128, EP // 128], BF16, kind="ExternalInput")
    AB_ds = [nc.dram_tensor(f"AB{c}", [cfg.CHUNK, 128], BF16)
             for c in range(cfg.C)]
    out_d = nc.dram_tensor("out", [cfg.NPC, 64], F32, kind="ExternalOutput")

    with TileContext(nc) as tc:
        nc.gpsimd.load_library(mlp_library)

        with tc.tile_pool(name="const", bufs=1) as cpool:
            waug_sb = cpool.tile([65, 128], BF16)
            nc.sync.dma_start(out=waug_sb[:], in_=waug_d[:])
            iota_sb = cpool.tile([128, 128], BF16)
            nc.sync.dma_start(out=iota_sb[:], in_=iota_d[:])
            ident_sb = cpool.tile([128, 128], BF16)
            nc.sync.dma_start(out=ident_sb[:], in_=ident_d[:])
            nw1_sb = cpool.tile([64, 64], BF16)
            nc.sync.dma_start(out=nw1_sb[:], in_=nw1_d[:])
            nw2a_sb = cpool.tile([65, 64], F32)
            nc.sync.dma_start(out=nw2a_sb[:], in_=nw2a_d[:])

            hTown_sb = cpool.tile([65, cfg.NPC], BF16)
            nc.sync.dma_start(out=hTown_sb[:], in_=hTown_d[:])

            aown_sb = cpool.tile([128, cfg.W, 64], BF16)
            arena = cpool.tile([65, cfg.NPC], F32)
            nc.vector.memset(arena[64:65, :], 1.0)

            # ---- Phase 0a: Aown into SBUF ----
            if 0 in phases:
                with tc.tile_pool(name="p0aps", bufs=4, space="PSUM") as p0aps:
                    for wb in range(cfg.W // 2):
                        ps = p0aps.tile([128, 2, 128], F32)
                        for i in range(2):
                            w = wb * 2 + i
                            nc.tensor.matmul(
                                out=ps[:, i, :],
                                lhsT=hTown_sb[:, w * 128:(w + 1) * 128],
                                rhs=waug_sb[:], start=True, stop=True)
                        if wb % 2 == 0:
                            nc.vector.tensor_copy(
                                out=aown_sb[:, wb * 2:wb * 2 + 2, :],
                                in_=ps[:, :, 0:64])
                        else:
                            nc.scalar.activation(
                                out=aown_sb[:, wb * 2:wb * 2 + 2, :],
                                in_=ps[:, :, 0:64],
                                func=mybir.ActivationFunctionType.Copy)

                # ---- Phase 0b: AB table (all NP nodes) to DRAM ----
                SN = cfg.stripe(cfg.CHUNK)
                JT = SN // 128
                with tc.tile_pool(name="p0", bufs=2) as p0, \
                     tc.tile_pool(name="p0ps", bufs=4, space="PSUM") as p0ps:
                    for st in range(cfg.NP // SN):
                        hstripe = p0.tile([65, SN], BF16, tag="hstripe")
                        nc.sync.dma_start(
                            out=hstripe[:], in_=hTa_d[:, st * SN:(st + 1) * SN])
                        abst = p0.tile([128, JT, 128], BF16, tag="abst")
                        for jb in range((JT + 3) // 4):
                            n = min(4, JT - jb * 4)
                            ps = p0ps.tile([128, 4, 128], F32)
                            for i in range(n):
                                j = jb * 4 + i
                                nc.tensor.matmul(
                                    out=ps[:, i, :],
                                    lhsT=hstripe[:, j * 128:(j + 1) * 128],
                                    rhs=waug_sb[:], start=True, stop=True)
                            if jb % 2 == 0:
                                nc.vector.tensor_copy(
                                    out=abst[:, jb * 4:jb * 4 + n, :],
                                    in_=ps[:, 0:n, :])
                            else:
                                nc.scalar.activation(
                                    out=abst[:, jb * 4:jb * 4 + n, :],
                                    in_=ps[:, 0:n, :],
                                    func=mybir.ActivationFunctionType.Copy)
                        n0 = st * SN
                        dst_d, off = AB_ds[n0 // cfg.CHUNK], n0 % cfg.CHUNK
                        # hTa columns are host-permuted so node off+p*JT+j is
                        # abst[p, j]; per-partition runs are contiguous
                        nc.sync.dma_start(
                            out=dst_d[off:off + SN, :].rearrange(
                                "(p j) f -> p j f", p=128),
                            in_=abst[:])

            # ---- Phase 1 + fused phase 2: skewed pipeline over segments ----
            if 1 in phases:
              with tc.tile_pool(name="rlp", bufs=1) as rlp:
                rl_sb = rlp.tile([128, EP // 128], BF16)
                nc.sync.dma_start(out=rl_sb[:], in_=rl_d[:])

                with tc.tile_pool(name="idxp", bufs=2) as idxp, \
                     tc.tile_pool(name="gath", bufs=6) as gathp, \
                     tc.tile_pool(name="ohp", bufs=6) as ohp, \
                     tc.tile_pool(name="ohtp", bufs=4) as ohtp, \
                     tc.tile_pool(name="mp", bufs=4) as mp, \
                     tc.tile_pool(name="p2o", bufs=2) as p2o, \
                     tc.tile_pool(name="tps", bufs=2, space="PSUM") as tps, \
                     tc.tile_pool(name="rvps", bufs=2, space="PSUM") as rvps, \
                     tc.tile_pool(name="aggps", bufs=2, space="PSUM") as aggps, \
                     tc.tile_pool(name="p2ps", bufs=1, space="PSUM") as p2ps:

                    cidx_tiles = {}
                    oh_tiles = {}          # seg -> tile
                    colg_tiles = {}        # seg -> tile
                    tb_tiles = {}          # batch -> psum tile
                    oht_tiles = {}         # batch -> sbuf tile
                    rv_tiles = {}          # batch -> psum tile
                    m2_tiles = {}          # batch -> sbuf tile
                    agg_tiles = {}         # window -> psum tile
                    n_T = [0]              # chunks transposed so far
                    n_rv = [0]             # chunks rv+madd'ed so far
                    n_relu = [0]           # batches relu'ed so far
                    ost_tiles = {}

                    def load_cidx(g):
                        if g * GSEG >= S:
                            return
                        n = min(GSEG, S - g * GSEG) * SEG // 16
                        t = idxp.tile([128, ILEN], I16, tag="cidx")
                        nc.sync.dma_start(
                            out=t[:, 0:n], in_=colidx_d[:, g * ILEN:g * ILEN + n])
                        cidx_tiles[g] = t

                    def gather(s):
                        g, r = divmod(s, GSEG)
                        t = gathp.tile([128, JPS, 128], BF16, tag="g")
                        nc.gpsimd.dma_gather(
                            t[:], AB_ds[s % cfg.C][:],
                            cidx_tiles[g][:, r * (SEG // 16):(r + 1) * (SEG // 16)],
                            SEG, SEG, 128, single_packet=SINGLE_PACKET,
                            queue_num=s % NSWQ)
                        colg_tiles[s] = t

                    def build_oh(s):
                        t = ohp.tile([128, JPS, 128], BF16, tag="oh")
                        nc.vector.tensor_tensor(
                            out=t[:],
                            in0=rl_sb[:, s * JPS:(s + 1) * JPS]
                                .to_broadcast([128, JPS, 128]),
                            in1=iota_sb[:].rearrange("p (a b) -> p a b", a=1)
                                .to_broadcast([128, JPS, 128]),
                            op=mybir.AluOpType.is_equal)
                        oh_tiles[s] = t

                    def transpose_upto(klim):
                        # transpose chunks [n_T, klim) into batch psum tiles;
                        # drain every completed batch (alternate DVE/ACT)
                        while n_T[0] < klim:
                            k = n_T[0]
                            b = k // 8
                            if k % 8 == 0:
                                tb_tiles[b] = tps.tile([128, 8, 128], BF16,
                                                       tag="tb", name="tb")
                            s, j = divmod(k, JPS)
                            nc.tensor.transpose(
                                out=tb_tiles[b][:, k % 8, :],
                                in_=oh_tiles[s][:, j, :],
                                identity=ident_sb[:])
                            n_T[0] += 1
                            if n_T[0] % 8 == 0 or n_T[0] == NCHUNK:
                                nb_ = 8 if n_T[0] % 8 == 0 else n_T[0] % 8
                                dst = ohtp.tile([128, 8, 128], BF16, tag="ohT")
                                if b % 2 == 0:
                                    nc.vector.tensor_copy(
                                        out=dst[:, 0:nb_, :],
                                        in_=tb_tiles[b][:, 0:nb_, :])
                                else:
                                    nc.scalar.activation(
                                        out=dst[:, 0:nb_, :],
                                        in_=tb_tiles[b][:, 0:nb_, :],
                                        func=mybir.ActivationFunctionType.Copy)
                                oht_tiles[b] = dst
                                tb_tiles.pop(b)

                    def rv_madd_seg(s):
                        # rv + colg accumulate for all chunks of segment s;
                        # relu every batch that becomes fully filled
                        w = s // cfg.C
                        for j in range(JPS):
                            k = s * JPS + j
                            b = k // 8
                            if k % 8 == 0:
                                rv_tiles[b] = rvps.tile([128, 8, 64], F32,
                                                        tag="rv", name="rv")
                            nc.tensor.matmul(
                                out=rv_tiles[b][:, k % 8, :],
                                lhsT=oht_tiles[b][:, k % 8, :],
                                rhs=aown_sb[:, w, :], start=True, stop=False)
                            nc.tensor.matmul(
                                out=rv_tiles[b][:, k % 8, :],
                                lhsT=ident_sb[:],
                                rhs=colg_tiles[s][:, j, 64:128],
                                start=False, stop=True)
                            n_rv[0] += 1
                        colg_tiles.pop(s)
                        while (n_relu[0] + 1) * 8 <= n_rv[0] or \
                              n_rv[0] == NCHUNK and n_relu[0] < NB:
                            b = n_relu[0]
                            nb_ = min(8, NCHUNK - b * 8)
                            dst = mp.tile([128, 8, 64], BF16, tag="m2")
                            nc.scalar.activation(
                                out=dst[:, 0:nb_, :],
                                in_=rv_tiles[b][:, 0:nb_, :],
                                func=mybir.ActivationFunctionType.Relu)
                            m2_tiles[b] = dst
                            rv_tiles.pop(b)
                            n_relu[0] += 1

                    def agg_seg(s):
                        w, c = divmod(s, cfg.C)
                        if c == 0:
                            agg_tiles[w] = aggps.tile([64, 128], F32, tag="agg", name="agg")
                        for j in range(JPS):
                            k = s * JPS + j
                            b = k // 8
                            nc.tensor.matmul(
                                out=agg_tiles[w][:],
                                lhsT=m2_tiles[b][:, k % 8, :],
                                rhs=oh_tiles[s][:, j, :],
                                start=(c == 0 and j == 0),
                                stop=(c == cfg.C - 1 and j == JPS - 1))
                        oh_tiles.pop(s)
                        if (s * JPS + JPS) % 8 != 0:
                            pass  # m2 batches span segments; popped lazily
                        for b in [bb for bb in m2_tiles
                                  if (bb + 1) * 8 <= s * JPS + JPS]:
                            m2_tiles.pop(b)
                        if c == cfg.C - 1:
                            flush_window(w)

                    def flush_window(w):
                        nc.vector.tensor_copy(
                            out=arena[0:64, w * 128:(w + 1) * 128],
                            in_=agg_tiles[w][:])
                        agg_tiles.pop(w)
                        if 2 in phases:
                            node_mlp(w)

                    def node_mlp(w):
                        if w % 2 == 0:
                            ost_tiles[w // 2] = p2o.tile([128, 2, 64], F32,
                                                         tag="ost", name="ost")
                        ps = p2ps.tile([128, 64], F32)
                        nc.tensor.matmul(
                            out=ps[:], lhsT=hTown_sb[0:64, w * 128:(w + 1) * 128],
                            rhs=nw1_sb[:], start=True, stop=False)
                        nc.tensor.matmul(
                            out=ps[:], lhsT=arena[:, w * 128:(w + 1) * 128],
                            rhs=nw2a_sb[:], start=False, stop=True)
                        nc.scalar.activation(
                            out=ost_tiles[w // 2][:, w % 2, :], in_=ps[:],
                            func=mybir.ActivationFunctionType.Relu)
                        if w % 2 == 1:
                            # row p*2+i of the block holds node (w-1+i)*128+p;
                            # unshard_output inverts this on host
                            nc.sync.dma_start(
                                out=out_d[(w - 1) * 128:(w + 1) * 128, :]
                                    .rearrange("(p i) f -> p i f", p=128),
                                in_=ost_tiles[w // 2][:])
                            ost_tiles.pop(w // 2)

                    # ---- the skewed main loop ----
                    load_cidx(0)
                    load_cidx(1)
                    for s in range(S + 3):
                        if s < S and s % GSEG == 0 and s > 0:
                            load_cidx(s // GSEG + 1)
                        if s < S:
                            gather(s)
                        if s < S:
                            build_oh(s)
                        if p1_level >= 2:
                            if s >= 1:
                                transpose_upto(min((s - 1 + 1) * JPS, NCHUNK))
                            if s >= 2 and s - 2 < S:
                                rv_madd_seg(s - 2)
                            if p1_level >= 4 and s >= 3 and s - 3 < S:
                                agg_seg(s - 3)

            elif 2 in phases:
                # standalone phase 2 (for ablation)
                with tc.tile_pool(name="p2", bufs=2) as p2, \
                     tc.tile_pool(name="p2ps", bufs=4, space="PSUM") as p2ps:
                    for g in range(cfg.W // 2):
                        ost = p2.tile([128, 2, 64], F32, tag="ost")
                        for i in range(2):
                            w = g * 2 + i
                            ps = p2ps.tile([128, 64], F32)
                            nc.tensor.matmul(
                                out=ps[:],
                                lhsT=hTown_sb[0:64, w * 128:(w + 1) * 128],
                                rhs=nw1_sb[:], start=True, stop=False)
                            nc.tensor.matmul(
                                out=ps[:], lhsT=arena[:, w * 128:(w + 1) * 128],
                                rhs=nw2a_sb[:], start=False, stop=True)
                            nc.scalar.activation(
                                out=ost[:, i, :], in_=ps[:],
                                func=mybir.ActivationFunctionType.Relu)
                        nc.sync.dma_start(
                            out=out_d[g * 256:(g + 1) * 256, :].rearrange(
                                "(p i) f -> p i f", p=128),
                            in_=ost[:])

    nc.compile()
    return nc


# ---------------- host-side data prep ----------------

def _wrap16(a):
    x = np.ascontiguousarray(a.reshape(-1, 16).T)
    return np.tile(x, (8, 1))


def _wrap128(a):
    return np.ascontiguousarray(a.reshape(-1, 128).T)


def prep_inputs(cfg, h, edge_index, edge_w, edge_b, node_w, node_b):
    N = cfg.N
    row = np.asarray(edge_index[0])
    col = np.asarray(edge_index[1])
    h = np.asarray(h, dtype=np.float32)

    hTa = np.zeros((65, cfg.NP), NP_BF16)
    hTa[:64, :N] = h.T.astype(NP_BF16)
    hTa[64, :] = 1.0
    # permute columns so p0b's natural (p, j) write order lands nodes
    # contiguously per partition: stripe col j*128+p <- node st*SN + p*JT + j
    SN = cfg.stripe(cfg.CHUNK)
    JT = SN // 128
    hTa_p = hTa.reshape(65, cfg.NP // SN, 128, JT).swapaxes(2, 3) \
        .reshape(65, cfg.NP)

    waug = np.zeros((65, 128), NP_BF16)
    waug[:64, 0:64] = edge_w[:64].astype(NP_BF16)
    waug[:64, 64:128] = edge_w[64:].astype(NP_BF16)
    waug[64, 0:64] = edge_b.astype(NP_BF16)

    nw1 = np.ascontiguousarray(node_w[:64]).astype(NP_BF16)
    nw2a = np.concatenate([node_w[64:], node_b[None, :]],
                          axis=0).astype(np.float32)

    iota = np.tile(np.arange(128, dtype=np.float32), (128, 1)).astype(NP_BF16)
    ident = np.eye(128, dtype=np.float32).astype(NP_BF16)

    # per-core edge prep; w-major segment order: seg_id = w*C + cc
    per_core = []
    maxc = 1
    for k in range(cfg.n_cores):
        base = k * cfg.NPC
        m = (row >= base) & (row < base + cfg.NPC)
        r = (row[m] - base).astype(np.int64)
        c = col[m].astype(np.int64)
        w = r >> 7
        cc = c // cfg.CHUNK
        seg_id = w * cfg.C + cc
        if cfg.col_sort:
            order = np.lexsort((c, seg_id))
        else:
            order = np.argsort(seg_id, kind="stable")
        r, c, seg_id = r[order], c[order], seg_id[order]
        counts = np.bincount(seg_id, minlength=cfg.C * cfg.W)
        if counts.size and r.size:
            maxc = max(maxc, int(counts.max()))
        per_core.append((r, c, seg_id, counts))
    SEG = int(math.ceil(maxc / 128.0)) * 128
    cfg.SEG = SEG
    EP = cfg.C * cfg.W * SEG

    in_maps = []
    for k in range(cfg.n_cores):
        r, c, seg_id, counts = per_core[k]
        starts = np.cumsum(counts) - counts
        intra = np.arange(r.size) - np.repeat(starts, counts)
        slots = seg_id * SEG + intra
        colidx = np.zeros(EP, np.int16)
        rl = np.full(EP, 255.0, NP_BF16)
        colidx[slots] = (c - (c // cfg.CHUNK) * cfg.CHUNK).astype(np.int16)
        rl[slots] = (r & 127).astype(NP_BF16)

        base = k * cfg.NPC
        hTown = np.ascontiguousarray(hTa[:, base:base + cfg.NPC])
        in_maps.append({
            "hTa": hTa_p,
            "hTown": hTown,
            "waug": waug,
            "nw1": nw1,
            "nw2a": nw2a,
            "iota": iota,
            "ident": ident,
            "colidx": _wrap16(colidx),
            "rl": _wrap128(rl),
        })
    return in_maps


def unshard_output(cfg, results):
    outs = []
    for res in results:
        o = np.asarray(res["out"])                      # [NPC, 64] permuted
        o = o.reshape(-1, 128, 2, 64).swapaxes(1, 2).reshape(-1, 64)
        outs.append(o)
    full = np.concatenate(outs, axis=0)
    return np.ascontiguousarray(full[:cfg.N]).astype(np.float32)


# ---------------- entry point ----------------

def kernel(h, edge_index, edge_w, edge_b, node_w, node_b):
    from concourse.bass_utils import run_bass_kernel_spmd
    cfg = Cfg(n_nodes=100000, n_cores=8)
    in_maps = prep_inputs(cfg, h, edge_index, edge_w, edge_b, node_w, node_b)
    nc = build_kernel(cfg)
    res = run_bass_kernel_spmd(nc, in_maps, core_ids=list(range(cfg.n_cores)))
    return unshard_output(cfg, res.results)
